# revision 19
# baseline (speedup 1.0000x reference)
"""Trainium2 Bass kernel for nn_CausalLinearSelfAttention_30013231464545.

Math note: the reference cumsums the [B,T,H,D,M] kv tensor over axis=-2,
which is the *D* axis (faithful to the original torch code), so
  kv_sum[b,t,h,d,m] = csD(kf)[b,t,h,d] * v[b,t,h,m]
and the whole module collapses to
  out[b,t,h,m] = (s / denom) * v[b,t,h,m]
with
  denom[b,t,h] = sum_d qf * cumsum_T(kf)      (true causal running key sum)
  s[b,t,h]     = sum_d qf * cumsum_D(kf)      (per-timestep D-prefix sum)
  qf = elu(q)+1 = min(exp(q), 1) + relu(q),  kf likewise.

Sharding: B*H = 16 (b,h) slices; each core takes one (b, head-pair) slice
[T=2048, 2*64] so DMA rows are 512B contiguous. No cross-core comm.

Per-core dataflow, v2 (T on partitions, (j,h,d) on free; fp16 on-chip):
  - 2 half-tensor chunks pipelined against the input DMA stream
  - feature maps: ACT exp (scalar) + fused (min 1) add relu via one
    scalar_tensor_tensor (DVE)
  - cumsum over T: per-tile triangular matmuls; the inter-tile carry runs
    entirely on the tensor engine (colsum matmuls onto partitions,
    one exclusive-tri16 matmul, rank-1 base adds) - no serial DVE chain
  - scalar engine copies PSUM ks -> SBUF f16 so the dot mults stay in
    DVE 2x fast mode
  - cumsum over D: one segmented scan per chunk (GpSimd, off the DVE)
  - dots over D: two f16 TT mults + one multi-axis tensor_reduce
  - out = v * (s/denom) broadcast multiply, DMA'd out per chunk
"""

import numpy as np
import sys

sys.path.insert(0, "/opt/trn_rl_repo")

B, T, H, D = 2, 2048, 8, 64
P = 128          # partitions (t per tile)
HPC = 2          # heads per core
C = HPC * D      # per-core free width = 128
NT = T // P      # 16 t-tiles per core
NCH = 2          # pipeline chunks
JT = NT // NCH   # 8 tiles per chunk
FD = JT * C      # 1024 free elements per chunk
NSEG = JT * HPC  # 16 (tile, head) segments per chunk

USE_GPSIMD_SCAN = False  # compiler rejects TensorTensorScan on the Pool engine

_CACHE = {}


def _build_nc():
    import concourse.bass as bass
    import concourse.bacc as bacc
    import concourse.mybir as mybir
    from concourse import tile

    dt = mybir.dt
    f32 = dt.float32
    f16 = dt.float16
    Alu = mybir.AluOpType
    Act = mybir.ActivationFunctionType

    nc = bacc.Bacc(None)

    q_d = nc.declare_dram_parameter("q", [T, C], f32, isOutput=False)
    k_d = nc.declare_dram_parameter("k", [T, C], f32, isOutput=False)
    v_d = nc.declare_dram_parameter("v", [T, C], f32, isOutput=False)
    o_d = nc.declare_dram_parameter("o", [T, C], f32, isOutput=True)

    # tri[t', t] = 1 if t' <= t  (lhsT for in-tile cumsum over partitions)
    tri_d = nc.inline_tensor(
        np.triu(np.ones((P, P), dtype=np.float16)), name="tri_const"
    )
    # gate2[p, (j, m)] = 1 if p < j: lhsT that sums colsum rows < j into
    # a broadcast over all 128 output partitions (the inter-tile base add)
    gate2_np = np.zeros((NT, NT, P), dtype=np.float16)
    for j in range(NT):
        gate2_np[:j, j, :] = 1.0
    gate2_d = nc.inline_tensor(gate2_np.reshape(NT, NT * P), name="gate2_const")
    # colsel[:, j, m] = (m == j): drops tile j's colsum onto psum row j
    colsel_np = np.zeros((P, JT, JT), dtype=np.float16)
    for j in range(JT):
        colsel_np[:, j, j] = 1.0
    colsel_d = nc.inline_tensor(colsel_np.reshape(P, JT * JT), name="colsel_const")

    with tile.TileContext(nc) as tc:
        with (
            tc.tile_pool(name="const", bufs=1) as cpool,
            tc.tile_pool(name="io", bufs=2) as io,
            tc.tile_pool(name="wk", bufs=2) as wk,
            tc.tile_pool(name="ps", bufs=2, space="PSUM") as pp,
            tc.tile_pool(name="pcs", bufs=1, space="PSUM") as pcs,
        ):
            # ---- DMA order: tri, then chunk-0 inputs, then the rest ----
            tri_t = cpool.tile([P, P], f16, tag="tri")
            nc.sync.dma_start(tri_t[:], tri_d[:])

            qtw = io.tile([P, T], f32, tag="q")
            ktw = io.tile([P, T], f32, tag="k")
            vtw = io.tile([P, T], f32, tag="v")

            def load_chunk(tw, d_, ch):
                rows = slice(ch * JT * P, (ch + 1) * JT * P)
                nc.sync.dma_start(
                    tw[:, ch * FD : (ch + 1) * FD].rearrange(
                        "p (j c) -> p j c", c=C
                    ),
                    d_[rows, :].rearrange("(j p) c -> p j c", p=P),
                )

            def load_qtr(eng, tw, d_, qtr):
                rows = slice(qtr * JT * P // 2, (qtr + 1) * JT * P // 2)
                fsl = slice(qtr * FD // 2, (qtr + 1) * FD // 2)
                eng.dma_start(
                    tw[:, fsl].rearrange("p (j c) -> p j c", c=C),
                    d_[rows, :].rearrange("(j p) c -> p j c", p=P),
                )

            # sync queue: k0 halves, consts, v; scalar queue: q0 halves, k1, q1
            load_qtr(nc.sync, ktw, k_d, 0)
            load_qtr(nc.sync, ktw, k_d, 1)
            load_qtr(nc.scalar, qtw, q_d, 0)
            load_qtr(nc.scalar, qtw, q_d, 1)
            colsel_t = cpool.tile([P, JT * JT], f16, tag="colsel")
            nc.sync.dma_start(colsel_t[:], colsel_d[:])
            gate2_t = cpool.tile([NT, NT * P], f16, tag="gate2")
            nc.sync.dma_start(gate2_t[:], gate2_d[:])
            load_qtr(nc.scalar, ktw, k_d, 2)
            load_qtr(nc.scalar, ktw, k_d, 3)
            load_qtr(nc.scalar, qtw, q_d, 2)
            load_qtr(nc.scalar, qtw, q_d, 3)
            load_chunk(vtw, v_d, 0)
            load_chunk(vtw, v_d, 1)
            # segmented-scan reset mask: 0 at the first column of each 64-seg
            mask_t = cpool.tile([P, FD], f16, tag="mask")
            nc.gpsimd.memset(mask_t[:], 1.0)
            nc.gpsimd.memset(
                mask_t[:].rearrange("p (s d) -> p s d", d=D)[:, :, 0:1], 0.0
            )
            # per-tile colsum rows for chunk>0, spread over partitions
            csm = cpool.tile([NT, C], f16, tag="csm")
            cst = [None] * NCH

            scan_eng = nc.gpsimd if USE_GPSIMD_SCAN else nc.vector

            for ch in range(NCH):
                gsl = slice(ch * FD, (ch + 1) * FD)
                kt = ktw[:, gsl]
                qt = qtw[:, gsl]
                vt = vtw[:, gsl]

                # ---- feature maps: f = min(exp(x),1) + relu(x) ----
                ek = wk.tile([P, FD], f16, tag="ek")
                nc.scalar.activation(ek[:], kt, Act.Exp)
                rk = wk.tile([P, FD], f16, tag="rk")
                nc.scalar.activation(rk[:], kt, Act.Relu)
                mek = wk.tile([P, FD], f16, tag="mek")
                nc.vector.tensor_scalar_min(mek[:], ek[:], 1.0)
                kf = wk.tile([P, FD], f16, tag="kf")
                nc.vector.tensor_tensor(kf[:], mek[:], rk[:], op=Alu.add)

                eq = wk.tile([P, FD], f16, tag="eq")
                nc.scalar.activation(eq[:], qt, Act.Exp)
                rq = wk.tile([P, FD], f16, tag="rq")
                nc.scalar.activation(rq[:], qt, Act.Relu)
                meq = wk.tile([P, FD], f16, tag="meq")
                nc.vector.tensor_scalar_min(meq[:], eq[:], 1.0)
                qf = wk.tile([P, FD], f16, tag="qf")
                nc.vector.tensor_tensor(qf[:], meq[:], rq[:], op=Alu.add)

                # ---- per-tile colsums onto psum rows 0..JT-1 ----
                cs_ps = pcs.tile([JT, C], f32, tag="cs")
                for j in range(JT):
                    nc.tensor.matmul(
                        cs_ps[:],
                        colsel_t[:, j * JT : (j + 1) * JT],
                        kf[:, j * C : (j + 1) * C],
                        start=(j == 0), stop=(j == JT - 1),
                    )
                cst_ch = wk.tile([JT, C], f16, tag=f"cst{ch}")
                cst[ch] = cst_ch
                nc.scalar.copy(cst_ch[:], cs_ps[:])
                # stage rows into csm (only chunk>0 reads it)
                nc.sync.dma_start(csm[ch * JT : (ch + 1) * JT, :], cst[ch][:])

                # ---- cumsum over T into PSUM: tri matmuls + per-tile
                # gated-colsum base matmuls (start=True resets the whole
                # PSUM bank, so only the first write per bank may set it)
                nk = (ch + 1) * JT
                cs_src = cst[0] if ch == 0 else csm
                ks = pp.tile([P, FD], f32, tag="ks")
                for j in range(JT):
                    sl = slice(j * C, (j + 1) * C)
                    nc.tensor.matmul(
                        ks[:, sl], tri_t[:], kf[:, sl],
                        start=(j % 4 == 0), stop=False,
                    )
                for j in range(JT):
                    jg = ch * JT + j
                    sl = slice(j * C, (j + 1) * C)
                    if jg == 0:
                        continue
                    nc.tensor.matmul(
                        ks[:, sl],
                        gate2_t[0:nk, jg * P : (jg + 1) * P],
                        cs_src[0:nk, :],
                        start=False,
                        stop=(j % 4 == 3),
                    )

                # ---- cumsum over D: one segmented scan ----
                csDm = wk.tile([P, FD], f16, tag="csDm")
                scan_eng.tensor_tensor_scan(
                    csDm[:], mask_t[:], kf[:], 0.0, op0=Alu.mult, op1=Alu.add
                )

                # ---- dots over D (s-part first: ready before ksf) ----
                scr = wk.tile([P, 2 * FD], f16, tag="scr")
                dn = wk.tile([P, 2 * NSEG], f32, tag="dn")
                nc.vector.tensor_tensor(
                    scr[:, FD : 2 * FD], qf[:], csDm[:], op=Alu.mult
                )
                nc.vector.tensor_tensor(
                    scr[:, 0:FD], qf[:], ks[:], op=Alu.mult
                )
                nc.vector.tensor_reduce(
                    dn[:],
                    scr[:].rearrange("p (s d) -> p s d", d=D),
                    axis=mybir.AxisListType.X,
                    op=Alu.add,
                )

                # scale = s / denom
                rec = wk.tile([P, NSEG], f32, tag="rec")
                nc.vector.reciprocal(rec[:], dn[:, 0:NSEG])
                sc = wk.tile([P, NSEG], f32, tag="sc")
                nc.vector.tensor_tensor(
                    sc[:], dn[:, NSEG : 2 * NSEG], rec[:], op=Alu.mult
                )

                # out = v * scale (broadcast over each 64-wide segment)
                ot = io.tile([P, FD], f32, tag="o")
                sc_b = sc[:].rearrange(
                    "p (s one) -> p s one", one=1
                ).broadcast_to([P, NSEG, D])
                nc.vector.tensor_tensor(
                    ot[:].rearrange("p (s d) -> p s d", d=D),
                    vt.rearrange("p (s d) -> p s d", d=D),
                    sc_b,
                    op=Alu.mult,
                )
                rows = slice(ch * JT * P, (ch + 1) * JT * P)
                ov = o_d[rows, :].rearrange("(j p) c -> p j c", p=P)
                nc.sync.dma_start(ov, ot[:].rearrange("p (j c) -> p j c", c=C))

    nc.compile()
    return nc


def get_nc():
    if "nc" not in _CACHE:
        _CACHE["nc"] = _build_nc()
    return _CACHE["nc"]


def shard_inputs(q, k, v):
    """core c -> (b = c//4, heads 2*(c%4), 2*(c%4)+1); returns list of in_maps."""
    maps = []
    for c in range(8):
        b, hp = divmod(c, 4)
        hs = slice(2 * hp, 2 * hp + 2)
        maps.append(
            {
                "q": np.ascontiguousarray(q[b, :, hs, :].reshape(T, C)),
                "k": np.ascontiguousarray(k[b, :, hs, :].reshape(T, C)),
                "v": np.ascontiguousarray(v[b, :, hs, :].reshape(T, C)),
            }
        )
    return maps


def gather_outputs(results):
    out = np.empty((B, T, H, D), dtype=np.float32)
    for c in range(8):
        b, hp = divmod(c, 4)
        out[b, :, 2 * hp : 2 * hp + 2, :] = results[c]["o"].reshape(T, HPC, D)
    return out


def kernel(q, k, v):
    from concourse.bass_utils import run_bass_kernel_spmd

    q = np.asarray(q, dtype=np.float32)
    k = np.asarray(k, dtype=np.float32)
    v = np.asarray(v, dtype=np.float32)
    nc = get_nc()
    maps = shard_inputs(q, k, v)
    res = run_bass_kernel_spmd(nc, maps, list(range(8)))
    return gather_outputs(res.results)


# revision 20
# speedup vs baseline: 1.0127x; 1.0127x over previous
"""Trainium2 Bass kernel for nn_CausalLinearSelfAttention_30013231464545.

Math note: the reference cumsums the [B,T,H,D,M] kv tensor over axis=-2,
which is the *D* axis (faithful to the original torch code), so
  kv_sum[b,t,h,d,m] = csD(kf)[b,t,h,d] * v[b,t,h,m]
and the whole module collapses to
  out[b,t,h,m] = (s / denom) * v[b,t,h,m]
with
  denom[b,t,h] = sum_d qf * cumsum_T(kf)      (true causal running key sum)
  s[b,t,h]     = sum_d qf * cumsum_D(kf)      (per-timestep D-prefix sum)
  qf = elu(q)+1 = min(exp(q), 1) + relu(q),  kf likewise.

Sharding: B*H = 16 (b,h) slices; each core takes one (b, head-pair) slice
[T=2048, 2*64] so DMA rows are 512B contiguous. No cross-core comm.

Per-core dataflow, v2 (T on partitions, (j,h,d) on free; fp16 on-chip):
  - 2 half-tensor chunks pipelined against the input DMA stream
  - feature maps: ACT exp (scalar) + fused (min 1) add relu via one
    scalar_tensor_tensor (DVE)
  - cumsum over T: per-tile triangular matmuls; the inter-tile carry runs
    entirely on the tensor engine (colsum matmuls onto partitions,
    one exclusive-tri16 matmul, rank-1 base adds) - no serial DVE chain
  - scalar engine copies PSUM ks -> SBUF f16 so the dot mults stay in
    DVE 2x fast mode
  - cumsum over D: one segmented scan per chunk (GpSimd, off the DVE)
  - dots over D: two f16 TT mults + one multi-axis tensor_reduce
  - out = v * (s/denom) broadcast multiply, DMA'd out per chunk
"""

import numpy as np
import sys

sys.path.insert(0, "/opt/trn_rl_repo")

B, T, H, D = 2, 2048, 8, 64
P = 128          # partitions (t per tile)
HPC = 2          # heads per core
C = HPC * D      # per-core free width = 128
NT = T // P      # 16 t-tiles per core
NCH = 2          # pipeline chunks
JT = NT // NCH   # 8 tiles per chunk
FD = JT * C      # 1024 free elements per chunk
NSEG = JT * HPC  # 16 (tile, head) segments per chunk

USE_GPSIMD_SCAN = False  # compiler rejects TensorTensorScan on the Pool engine

_CACHE = {}


def _build_nc():
    import concourse.bass as bass
    import concourse.bacc as bacc
    import concourse.mybir as mybir
    from concourse import tile

    dt = mybir.dt
    f32 = dt.float32
    f16 = dt.float16
    Alu = mybir.AluOpType
    Act = mybir.ActivationFunctionType

    nc = bacc.Bacc(None)

    q_d = nc.declare_dram_parameter("q", [T, C], f32, isOutput=False)
    k_d = nc.declare_dram_parameter("k", [T, C], f32, isOutput=False)
    v_d = nc.declare_dram_parameter("v", [T, C], f32, isOutput=False)
    o_d = nc.declare_dram_parameter("o", [T, C], f32, isOutput=True)

    # tri[t', t] = 1 if t' <= t  (lhsT for in-tile cumsum over partitions)
    tri_d = nc.inline_tensor(
        np.triu(np.ones((P, P), dtype=np.float16)), name="tri_const"
    )
    # gate2[p, (j, m)] = 1 if p < j: lhsT that sums colsum rows < j into
    # a broadcast over all 128 output partitions (the inter-tile base add)
    gate2_np = np.zeros((NT, NT, P), dtype=np.float16)
    for j in range(NT):
        gate2_np[:j, j, :] = 1.0
    gate2_d = nc.inline_tensor(gate2_np.reshape(NT, NT * P), name="gate2_const")
    # colsel[:, j, m] = (m == j): drops tile j's colsum onto psum row j
    colsel_np = np.zeros((P, JT, JT), dtype=np.float16)
    for j in range(JT):
        colsel_np[:, j, j] = 1.0
    colsel_d = nc.inline_tensor(colsel_np.reshape(P, JT * JT), name="colsel_const")

    with tile.TileContext(nc) as tc:
        with (
            tc.tile_pool(name="const", bufs=1) as cpool,
            tc.tile_pool(name="io", bufs=2) as io,
            tc.tile_pool(name="wk", bufs=2) as wk,
            tc.tile_pool(name="ps", bufs=2, space="PSUM") as pp,
            tc.tile_pool(name="pcs", bufs=1, space="PSUM") as pcs,
        ):
            # ---- DMA order: tri, then chunk-0 inputs, then the rest ----
            tri_t = cpool.tile([P, P], f16, tag="tri")
            nc.sync.dma_start(tri_t[:], tri_d[:])

            qtw = io.tile([P, T], f32, tag="q")
            ktw = io.tile([P, T], f32, tag="k")
            vtw = io.tile([P, T], f32, tag="v")

            def load_chunk(tw, d_, ch):
                rows = slice(ch * JT * P, (ch + 1) * JT * P)
                nc.sync.dma_start(
                    tw[:, ch * FD : (ch + 1) * FD].rearrange(
                        "p (j c) -> p j c", c=C
                    ),
                    d_[rows, :].rearrange("(j p) c -> p j c", p=P),
                )

            def load_qtr(eng, tw, d_, qtr):
                rows = slice(qtr * JT * P // 2, (qtr + 1) * JT * P // 2)
                fsl = slice(qtr * FD // 2, (qtr + 1) * FD // 2)
                eng.dma_start(
                    tw[:, fsl].rearrange("p (j c) -> p j c", c=C),
                    d_[rows, :].rearrange("(j p) c -> p j c", p=P),
                )

            # serial issue on one queue, in consumption order: the issue
            # rate throttles later streams so k0/q0 finish first
            load_qtr(nc.sync, ktw, k_d, 0)
            load_qtr(nc.sync, ktw, k_d, 1)
            load_qtr(nc.sync, qtw, q_d, 0)
            load_qtr(nc.sync, qtw, q_d, 1)
            colsel_t = cpool.tile([P, JT * JT], f16, tag="colsel")
            nc.sync.dma_start(colsel_t[:], colsel_d[:])
            gate2_t = cpool.tile([NT, NT * P], f16, tag="gate2")
            nc.sync.dma_start(gate2_t[:], gate2_d[:])
            load_qtr(nc.sync, ktw, k_d, 2)
            load_qtr(nc.sync, ktw, k_d, 3)
            load_qtr(nc.sync, qtw, q_d, 2)
            load_qtr(nc.sync, qtw, q_d, 3)
            load_chunk(vtw, v_d, 0)
            load_chunk(vtw, v_d, 1)
            # segmented-scan reset mask: 0 at the first column of each 64-seg
            mask_t = cpool.tile([P, FD], f16, tag="mask")
            nc.gpsimd.memset(mask_t[:], 1.0)
            nc.gpsimd.memset(
                mask_t[:].rearrange("p (s d) -> p s d", d=D)[:, :, 0:1], 0.0
            )
            # per-tile colsum rows for chunk>0, spread over partitions
            csm = cpool.tile([NT, C], f16, tag="csm")
            cst = [None] * NCH

            scan_eng = nc.gpsimd if USE_GPSIMD_SCAN else nc.vector

            for ch in range(NCH):
                gsl = slice(ch * FD, (ch + 1) * FD)
                kt = ktw[:, gsl]
                qt = qtw[:, gsl]
                vt = vtw[:, gsl]

                # ---- feature maps: f = min(exp(x),1) + relu(x) ----
                ek = wk.tile([P, FD], f16, tag="ek")
                nc.scalar.activation(ek[:], kt, Act.Exp)
                rk = wk.tile([P, FD], f16, tag="rk")
                nc.scalar.activation(rk[:], kt, Act.Relu)
                mek = wk.tile([P, FD], f16, tag="mek")
                nc.vector.tensor_scalar_min(mek[:], ek[:], 1.0)
                kf = wk.tile([P, FD], f16, tag="kf")
                nc.vector.tensor_tensor(kf[:], mek[:], rk[:], op=Alu.add)

                eq = wk.tile([P, FD], f16, tag="eq")
                nc.scalar.activation(eq[:], qt, Act.Exp)
                rq = wk.tile([P, FD], f16, tag="rq")
                nc.scalar.activation(rq[:], qt, Act.Relu)
                meq = wk.tile([P, FD], f16, tag="meq")
                nc.vector.tensor_scalar_min(meq[:], eq[:], 1.0)
                qf = wk.tile([P, FD], f16, tag="qf")
                nc.vector.tensor_tensor(qf[:], meq[:], rq[:], op=Alu.add)

                # ---- per-tile colsums onto psum rows 0..JT-1 ----
                cs_ps = pcs.tile([JT, C], f32, tag="cs")
                for j in range(JT):
                    nc.tensor.matmul(
                        cs_ps[:],
                        colsel_t[:, j * JT : (j + 1) * JT],
                        kf[:, j * C : (j + 1) * C],
                        start=(j == 0), stop=(j == JT - 1),
                    )
                cst_ch = wk.tile([JT, C], f16, tag=f"cst{ch}")
                cst[ch] = cst_ch
                nc.scalar.copy(cst_ch[:], cs_ps[:])
                # stage rows into csm (only chunk>0 reads it)
                nc.sync.dma_start(csm[ch * JT : (ch + 1) * JT, :], cst[ch][:])

                # ---- cumsum over T into PSUM: tri matmuls + per-tile
                # gated-colsum base matmuls (start=True resets the whole
                # PSUM bank, so only the first write per bank may set it)
                nk = (ch + 1) * JT
                cs_src = cst[0] if ch == 0 else csm
                ks = pp.tile([P, FD], f32, tag="ks")
                for j in range(JT):
                    sl = slice(j * C, (j + 1) * C)
                    nc.tensor.matmul(
                        ks[:, sl], tri_t[:], kf[:, sl],
                        start=(j % 4 == 0), stop=False,
                    )
                for j in range(JT):
                    jg = ch * JT + j
                    sl = slice(j * C, (j + 1) * C)
                    if jg == 0:
                        continue
                    nc.tensor.matmul(
                        ks[:, sl],
                        gate2_t[0:nk, jg * P : (jg + 1) * P],
                        cs_src[0:nk, :],
                        start=False,
                        stop=(j % 4 == 3),
                    )

                # ---- cumsum over D: one segmented scan ----
                csDm = wk.tile([P, FD], f16, tag="csDm")
                scan_eng.tensor_tensor_scan(
                    csDm[:], mask_t[:], kf[:], 0.0, op0=Alu.mult, op1=Alu.add
                )

                # ---- dots over D (s-part first: ready before ks) ----
                scr = wk.tile([P, 2 * FD], f16, tag="scr")
                dn = wk.tile([P, 2 * NSEG], f32, tag="dn")
                nc.vector.tensor_tensor(
                    scr[:, FD : 2 * FD], qf[:], csDm[:], op=Alu.mult
                )
                nc.vector.tensor_reduce(
                    dn[:, NSEG : 2 * NSEG],
                    scr[:, FD : 2 * FD].rearrange("p (s d) -> p s d", d=D),
                    axis=mybir.AxisListType.X,
                    op=Alu.add,
                )
                nc.vector.tensor_tensor(
                    scr[:, 0:FD], qf[:], ks[:], op=Alu.mult
                )
                nc.vector.tensor_reduce(
                    dn[:, 0:NSEG],
                    scr[:, 0:FD].rearrange("p (s d) -> p s d", d=D),
                    axis=mybir.AxisListType.X,
                    op=Alu.add,
                )

                # scale = s / denom
                rec = wk.tile([P, NSEG], f32, tag="rec")
                nc.vector.reciprocal(rec[:], dn[:, 0:NSEG])
                sc = wk.tile([P, NSEG], f32, tag="sc")
                nc.vector.tensor_tensor(
                    sc[:], dn[:, NSEG : 2 * NSEG], rec[:], op=Alu.mult
                )

                # out = v * scale (broadcast over each 64-wide segment),
                # in halves so the store DMA overlaps the second half
                ot = io.tile([P, FD], f32, tag="o")
                for half in range(2):
                    hs = slice(half * FD // 2, (half + 1) * FD // 2)
                    sseg = slice(half * NSEG // 2, (half + 1) * NSEG // 2)
                    sc_b = sc[:, sseg].rearrange(
                        "p (s one) -> p s one", one=1
                    ).broadcast_to([P, NSEG // 2, D])
                    nc.vector.tensor_tensor(
                        ot[:, hs].rearrange("p (s d) -> p s d", d=D),
                        vt[:, hs].rearrange("p (s d) -> p s d", d=D),
                        sc_b,
                        op=Alu.mult,
                    )
                    qtr = ch * 2 + half
                    rows = slice(qtr * JT * P // 2, (qtr + 1) * JT * P // 2)
                    ov = o_d[rows, :].rearrange("(j p) c -> p j c", p=P)
                    nc.sync.dma_start(
                        ov, ot[:, hs].rearrange("p (j c) -> p j c", c=C)
                    )

    nc.compile()
    return nc


def get_nc():
    if "nc" not in _CACHE:
        _CACHE["nc"] = _build_nc()
    return _CACHE["nc"]


def shard_inputs(q, k, v):
    """core c -> (b = c//4, heads 2*(c%4), 2*(c%4)+1); returns list of in_maps."""
    maps = []
    for c in range(8):
        b, hp = divmod(c, 4)
        hs = slice(2 * hp, 2 * hp + 2)
        maps.append(
            {
                "q": np.ascontiguousarray(q[b, :, hs, :].reshape(T, C)),
                "k": np.ascontiguousarray(k[b, :, hs, :].reshape(T, C)),
                "v": np.ascontiguousarray(v[b, :, hs, :].reshape(T, C)),
            }
        )
    return maps


def gather_outputs(results):
    out = np.empty((B, T, H, D), dtype=np.float32)
    for c in range(8):
        b, hp = divmod(c, 4)
        out[b, :, 2 * hp : 2 * hp + 2, :] = results[c]["o"].reshape(T, HPC, D)
    return out


def kernel(q, k, v):
    from concourse.bass_utils import run_bass_kernel_spmd

    q = np.asarray(q, dtype=np.float32)
    k = np.asarray(k, dtype=np.float32)
    v = np.asarray(v, dtype=np.float32)
    nc = get_nc()
    maps = shard_inputs(q, k, v)
    res = run_bass_kernel_spmd(nc, maps, list(range(8)))
    return gather_outputs(res.results)


# revision 21
# speedup vs baseline: 1.0236x; 1.0108x over previous
"""Trainium2 Bass kernel for nn_CausalLinearSelfAttention_30013231464545.

Math note: the reference cumsums the [B,T,H,D,M] kv tensor over axis=-2,
which is the *D* axis (faithful to the original torch code), so
  kv_sum[b,t,h,d,m] = csD(kf)[b,t,h,d] * v[b,t,h,m]
and the whole module collapses to
  out[b,t,h,m] = (s / denom) * v[b,t,h,m]
with
  denom[b,t,h] = sum_d qf * cumsum_T(kf)      (true causal running key sum)
  s[b,t,h]     = sum_d qf * cumsum_D(kf)      (per-timestep D-prefix sum)
  qf = elu(q)+1 = min(exp(q), 1) + relu(q),  kf likewise.

Sharding: B*H = 16 (b,h) slices; each core takes one (b, head-pair) slice
[T=2048, 2*64] so DMA rows are 512B contiguous. No cross-core comm.

Per-core dataflow, v2 (T on partitions, (j,h,d) on free; fp16 on-chip):
  - 2 half-tensor chunks pipelined against the input DMA stream
  - feature maps: ACT exp (scalar) + fused (min 1) add relu via one
    scalar_tensor_tensor (DVE)
  - cumsum over T: per-tile triangular matmuls; the inter-tile carry runs
    entirely on the tensor engine (colsum matmuls onto partitions,
    one exclusive-tri16 matmul, rank-1 base adds) - no serial DVE chain
  - scalar engine copies PSUM ks -> SBUF f16 so the dot mults stay in
    DVE 2x fast mode
  - cumsum over D: one segmented scan per chunk (GpSimd, off the DVE)
  - dots over D: two f16 TT mults + one multi-axis tensor_reduce
  - out = v * (s/denom) broadcast multiply, DMA'd out per chunk
"""

import numpy as np
import sys

sys.path.insert(0, "/opt/trn_rl_repo")

B, T, H, D = 2, 2048, 8, 64
P = 128          # partitions (t per tile)
HPC = 2          # heads per core
C = HPC * D      # per-core free width = 128
NT = T // P      # 16 t-tiles per core
NCH = 2          # pipeline chunks
JT = NT // NCH   # 8 tiles per chunk
FD = JT * C      # 1024 free elements per chunk
NSEG = JT * HPC  # 16 (tile, head) segments per chunk

USE_GPSIMD_SCAN = False  # compiler rejects TensorTensorScan on the Pool engine

_CACHE = {}


def _build_nc():
    import concourse.bass as bass
    import concourse.bacc as bacc
    import concourse.mybir as mybir
    from concourse import tile

    dt = mybir.dt
    f32 = dt.float32
    f16 = dt.float16
    Alu = mybir.AluOpType
    Act = mybir.ActivationFunctionType

    nc = bacc.Bacc(None)

    q_d = nc.declare_dram_parameter("q", [T, C], f32, isOutput=False)
    k_d = nc.declare_dram_parameter("k", [T, C], f32, isOutput=False)
    v_d = nc.declare_dram_parameter("v", [T, C], f32, isOutput=False)
    o_d = nc.declare_dram_parameter("o", [T, C], f32, isOutput=True)

    # tri[t', t] = 1 if t' <= t  (lhsT for in-tile cumsum over partitions)
    tri_d = nc.inline_tensor(
        np.triu(np.ones((P, P), dtype=np.float16)), name="tri_const"
    )
    # gate2[p, (j, m)] = 1 if p < j: lhsT that sums colsum rows < j into
    # a broadcast over all 128 output partitions (the inter-tile base add)
    gate2_np = np.zeros((NT, NT, P), dtype=np.float16)
    for j in range(NT):
        gate2_np[:j, j, :] = 1.0
    gate2_d = nc.inline_tensor(gate2_np.reshape(NT, NT * P), name="gate2_const")
    # colsel[:, j, m] = (m == j): drops tile j's colsum onto psum row j
    colsel_np = np.zeros((P, JT, JT), dtype=np.float16)
    for j in range(JT):
        colsel_np[:, j, j] = 1.0
    colsel_d = nc.inline_tensor(colsel_np.reshape(P, JT * JT), name="colsel_const")

    with tile.TileContext(nc) as tc:
        with (
            tc.tile_pool(name="const", bufs=1) as cpool,
            tc.tile_pool(name="io", bufs=2) as io,
            tc.tile_pool(name="wk", bufs=2) as wk,
            tc.tile_pool(name="ps", bufs=2, space="PSUM") as pp,
            tc.tile_pool(name="pcs", bufs=1, space="PSUM") as pcs,
        ):
            qtw = io.tile([P, T], f32, tag="q")
            ktw = io.tile([P, T], f32, tag="k")
            vtw = io.tile([P, T], f32, tag="v")

            def load_qtr(eng, tw, d_, qtr):
                rows = slice(qtr * JT * P // 2, (qtr + 1) * JT * P // 2)
                fsl = slice(qtr * FD // 2, (qtr + 1) * FD // 2)
                eng.dma_start(
                    tw[:, fsl].rearrange("p (j c) -> p j c", c=C),
                    d_[rows, :].rearrange("(j p) c -> p j c", p=P),
                )

            # k0 first on sync, q0 on scalar: these two finish before the
            # rest of the input stream starts competing for bandwidth.
            load_qtr(nc.sync, ktw, k_d, 0)
            load_qtr(nc.sync, ktw, k_d, 1)
            load_qtr(nc.scalar, qtw, q_d, 0)
            load_qtr(nc.scalar, qtw, q_d, 1)
            tri_t = cpool.tile([P, P], f16, tag="tri")
            nc.sync.dma_start(tri_t[:], tri_d[:])
            colsel_t = cpool.tile([P, JT * JT], f16, tag="colsel")
            nc.sync.dma_start(colsel_t[:], colsel_d[:])
            gate2_t = cpool.tile([NT, NT * P], f16, tag="gate2")
            nc.sync.dma_start(gate2_t[:], gate2_d[:])
            load_qtr(nc.sync, vtw, v_d, 0)
            load_qtr(nc.sync, vtw, v_d, 1)
            load_qtr(nc.sync, vtw, v_d, 2)
            load_qtr(nc.sync, vtw, v_d, 3)

            # segmented-scan reset mask: 0 at the first column of each 64-seg
            mask_t = cpool.tile([P, FD], f16, tag="mask")
            nc.gpsimd.memset(mask_t[:], 1.0)
            nc.gpsimd.memset(
                mask_t[:].rearrange("p (s d) -> p s d", d=D)[:, :, 0:1], 0.0
            )
            # per-tile colsum rows for chunk>0, spread over partitions
            csm = cpool.tile([NT, C], f16, tag="csm")
            cst = [None] * NCH

            for ch in range(NCH):
                gsl = slice(ch * FD, (ch + 1) * FD)
                kt = ktw[:, gsl]
                qt = qtw[:, gsl]
                vt = vtw[:, gsl]

                # ---- feature maps: f = min(exp(x),1) + relu(x) ----
                # (exp+relu on the scalar engine; fused min+add on DVE)
                ek = wk.tile([P, FD], f16, tag="ek")
                nc.scalar.activation(ek[:], kt, Act.Exp)
                rk = wk.tile([P, FD], f16, tag="rk")
                nc.scalar.activation(rk[:], kt, Act.Relu)
                if ch == 0:
                    # later input streams issue only now, so chunk-0 data
                    # wins the DMA bandwidth race
                    load_qtr(nc.scalar, ktw, k_d, 2)
                    load_qtr(nc.scalar, ktw, k_d, 3)
                kf = wk.tile([P, FD], f16, tag="kf")
                nc.vector.scalar_tensor_tensor(
                    kf[:], ek[:], 1.0, rk[:], op0=Alu.min, op1=Alu.add
                )

                eq = wk.tile([P, FD], f16, tag="eq")
                nc.scalar.activation(eq[:], qt, Act.Exp)
                rq = wk.tile([P, FD], f16, tag="rq")
                nc.scalar.activation(rq[:], qt, Act.Relu)
                if ch == 0:
                    load_qtr(nc.scalar, qtw, q_d, 2)
                    load_qtr(nc.scalar, qtw, q_d, 3)
                qf = wk.tile([P, FD], f16, tag="qf")
                nc.vector.scalar_tensor_tensor(
                    qf[:], eq[:], 1.0, rq[:], op0=Alu.min, op1=Alu.add
                )

                # ---- per-tile colsums onto psum rows 0..JT-1 ----
                cs_ps = pcs.tile([JT, C], f32, tag="cs")
                for j in range(JT):
                    nc.tensor.matmul(
                        cs_ps[:],
                        colsel_t[:, j * JT : (j + 1) * JT],
                        kf[:, j * C : (j + 1) * C],
                        start=(j == 0), stop=(j == JT - 1),
                    )
                cst_ch = wk.tile([JT, C], f16, tag=f"cst{ch}")
                cst[ch] = cst_ch
                nc.scalar.copy(cst_ch[:], cs_ps[:])
                # stage rows into csm (only chunk>0 reads it)
                nc.sync.dma_start(csm[ch * JT : (ch + 1) * JT, :], cst_ch[:])

                # ---- cumsum over T into PSUM: tri matmuls + per-tile
                # gated-colsum base matmuls (start=True resets the whole
                # PSUM bank, so only the first write per bank may set it)
                nk = (ch + 1) * JT
                cs_src = cst[0] if ch == 0 else csm
                ks = pp.tile([P, FD], f32, tag="ks")
                for j in range(JT):
                    sl = slice(j * C, (j + 1) * C)
                    nc.tensor.matmul(
                        ks[:, sl], tri_t[:], kf[:, sl],
                        start=(j % 4 == 0), stop=False,
                    )
                for j in range(JT):
                    jg = ch * JT + j
                    sl = slice(j * C, (j + 1) * C)
                    if jg == 0:
                        continue
                    nc.tensor.matmul(
                        ks[:, sl],
                        gate2_t[0:nk, jg * P : (jg + 1) * P],
                        cs_src[0:nk, :],
                        start=False,
                        stop=(j % 4 == 3),
                    )

                # ---- cumsum over D: one segmented scan ----
                csDm = wk.tile([P, FD], f16, tag="csDm")
                nc.vector.tensor_tensor_scan(
                    csDm[:], mask_t[:], kf[:], 0.0, op0=Alu.mult, op1=Alu.add
                )

                # ---- dots over D (s-part first: ready before ks) ----
                scr = wk.tile([P, 2 * FD], f16, tag="scr")
                dn = wk.tile([P, 2 * NSEG], f32, tag="dn")
                nc.vector.tensor_tensor(
                    scr[:, FD : 2 * FD], qf[:], csDm[:], op=Alu.mult
                )
                nc.vector.tensor_reduce(
                    dn[:, NSEG : 2 * NSEG],
                    scr[:, FD : 2 * FD].rearrange("p (s d) -> p s d", d=D),
                    axis=mybir.AxisListType.X,
                    op=Alu.add,
                )
                nc.vector.tensor_tensor(
                    scr[:, 0:FD], qf[:], ks[:], op=Alu.mult
                )
                nc.vector.tensor_reduce(
                    dn[:, 0:NSEG],
                    scr[:, 0:FD].rearrange("p (s d) -> p s d", d=D),
                    axis=mybir.AxisListType.X,
                    op=Alu.add,
                )

                # scale = s / denom
                rec = wk.tile([P, NSEG], f32, tag="rec")
                nc.vector.reciprocal(rec[:], dn[:, 0:NSEG])
                sc = wk.tile([P, NSEG], f32, tag="sc")
                nc.vector.tensor_tensor(
                    sc[:], dn[:, NSEG : 2 * NSEG], rec[:], op=Alu.mult
                )

                # out = v * scale (broadcast over each 64-wide segment),
                # in halves so the store DMA overlaps the second half
                ot = io.tile([P, FD], f32, tag="o")
                for half in range(2):
                    hs = slice(half * FD // 2, (half + 1) * FD // 2)
                    sseg = slice(half * NSEG // 2, (half + 1) * NSEG // 2)
                    sc_b = sc[:, sseg].rearrange(
                        "p (s one) -> p s one", one=1
                    ).broadcast_to([P, NSEG // 2, D])
                    nc.vector.tensor_tensor(
                        ot[:, hs].rearrange("p (s d) -> p s d", d=D),
                        vt[:, hs].rearrange("p (s d) -> p s d", d=D),
                        sc_b,
                        op=Alu.mult,
                    )
                    qtr = ch * 2 + half
                    rows = slice(qtr * JT * P // 2, (qtr + 1) * JT * P // 2)
                    ov = o_d[rows, :].rearrange("(j p) c -> p j c", p=P)
                    nc.sync.dma_start(
                        ov, ot[:, hs].rearrange("p (j c) -> p j c", c=C)
                    )

    nc.compile()
    return nc


def get_nc():
    if "nc" not in _CACHE:
        _CACHE["nc"] = _build_nc()
    return _CACHE["nc"]


def shard_inputs(q, k, v):
    """core c -> (b = c//4, heads 2*(c%4), 2*(c%4)+1); returns list of in_maps."""
    maps = []
    for c in range(8):
        b, hp = divmod(c, 4)
        hs = slice(2 * hp, 2 * hp + 2)
        maps.append(
            {
                "q": np.ascontiguousarray(q[b, :, hs, :].reshape(T, C)),
                "k": np.ascontiguousarray(k[b, :, hs, :].reshape(T, C)),
                "v": np.ascontiguousarray(v[b, :, hs, :].reshape(T, C)),
            }
        )
    return maps


def gather_outputs(results):
    out = np.empty((B, T, H, D), dtype=np.float32)
    for c in range(8):
        b, hp = divmod(c, 4)
        out[b, :, 2 * hp : 2 * hp + 2, :] = results[c]["o"].reshape(T, HPC, D)
    return out


def kernel(q, k, v):
    from concourse.bass_utils import run_bass_kernel_spmd

    q = np.asarray(q, dtype=np.float32)
    k = np.asarray(k, dtype=np.float32)
    v = np.asarray(v, dtype=np.float32)
    nc = get_nc()
    maps = shard_inputs(q, k, v)
    res = run_bass_kernel_spmd(nc, maps, list(range(8)))
    return gather_outputs(res.results)


# revision 22
# speedup vs baseline: 1.0452x; 1.0211x over previous
"""Trainium2 Bass kernel for nn_CausalLinearSelfAttention_30013231464545.

Math note: the reference cumsums the [B,T,H,D,M] kv tensor over axis=-2,
which is the *D* axis (faithful to the original torch code), so
  kv_sum[b,t,h,d,m] = csD(kf)[b,t,h,d] * v[b,t,h,m]
and the whole module collapses to
  out[b,t,h,m] = (s / denom) * v[b,t,h,m]
with
  denom[b,t,h] = sum_d qf * cumsum_T(kf)      (true causal running key sum)
  s[b,t,h]     = sum_d qf * cumsum_D(kf)      (per-timestep D-prefix sum)
  qf = elu(q)+1 = min(exp(q), 1) + relu(q),  kf likewise.

Sharding: B*H = 16 (b,h) slices; each core takes one (b, head-pair) slice
[T=2048, 2*64] so DMA rows are 512B contiguous. No cross-core comm.

Per-core dataflow, v2 (T on partitions, (j,h,d) on free; fp16 on-chip):
  - 2 half-tensor chunks pipelined against the input DMA stream
  - feature maps: ACT exp (scalar) + fused (min 1) add relu via one
    scalar_tensor_tensor (DVE)
  - cumsum over T: per-tile triangular matmuls; the inter-tile carry runs
    entirely on the tensor engine (colsum matmuls onto partitions,
    one exclusive-tri16 matmul, rank-1 base adds) - no serial DVE chain
  - scalar engine copies PSUM ks -> SBUF f16 so the dot mults stay in
    DVE 2x fast mode
  - cumsum over D: one segmented scan per chunk (GpSimd, off the DVE)
  - dots over D: two f16 TT mults + one multi-axis tensor_reduce
  - out = v * (s/denom) broadcast multiply, DMA'd out per chunk
"""

import numpy as np
import sys

sys.path.insert(0, "/opt/trn_rl_repo")

B, T, H, D = 2, 2048, 8, 64
P = 128          # partitions (t per tile)
HPC = 2          # heads per core
C = HPC * D      # per-core free width = 128
NT = T // P      # 16 t-tiles per core
NCH = 2          # pipeline chunks
JT = NT // NCH   # 8 tiles per chunk
FD = JT * C      # 1024 free elements per chunk
NSEG = JT * HPC  # 16 (tile, head) segments per chunk

USE_GPSIMD_SCAN = False  # compiler rejects TensorTensorScan on the Pool engine

_CACHE = {}


def _build_nc():
    import concourse.bass as bass
    import concourse.bacc as bacc
    import concourse.mybir as mybir
    from concourse import tile

    dt = mybir.dt
    f32 = dt.float32
    f16 = dt.float16
    Alu = mybir.AluOpType
    Act = mybir.ActivationFunctionType

    nc = bacc.Bacc(None)

    q_d = nc.declare_dram_parameter("q", [T, C], f32, isOutput=False)
    k_d = nc.declare_dram_parameter("k", [T, C], f32, isOutput=False)
    v_d = nc.declare_dram_parameter("v", [T, C], f32, isOutput=False)
    o_d = nc.declare_dram_parameter("o", [T, C], f32, isOutput=True)

    # tri[t', t] = 1 if t' <= t  (lhsT for in-tile cumsum over partitions)
    tri_d = nc.inline_tensor(
        np.triu(np.ones((P, P), dtype=np.float16)), name="tri_const"
    )
    # gate2[p, (j, m)] = 1 if p < j: lhsT that sums colsum rows < j into
    # a broadcast over all 128 output partitions (the inter-tile base add)
    gate2_np = np.zeros((NT, NT, P), dtype=np.float16)
    for j in range(NT):
        gate2_np[:j, j, :] = 1.0
    gate2_d = nc.inline_tensor(gate2_np.reshape(NT, NT * P), name="gate2_const")
    # colsel[:, j, m] = (m == j): drops tile j's colsum onto psum row j
    colsel_np = np.zeros((P, JT, JT), dtype=np.float16)
    for j in range(JT):
        colsel_np[:, j, j] = 1.0
    colsel_d = nc.inline_tensor(colsel_np.reshape(P, JT * JT), name="colsel_const")

    with tile.TileContext(nc) as tc:
        with (
            tc.tile_pool(name="const", bufs=1) as cpool,
            tc.tile_pool(name="io", bufs=2) as io,
            tc.tile_pool(name="wk", bufs=2) as wk,
            tc.tile_pool(name="ps", bufs=2, space="PSUM") as pp,
            tc.tile_pool(name="pcs", bufs=1, space="PSUM") as pcs,
        ):
            qtw = io.tile([P, T], f32, tag="q")
            ktw = io.tile([P, T], f32, tag="k")
            vtw = io.tile([P, T], f32, tag="v")

            def load_qtr(eng, tw, d_, qtr):
                rows = slice(qtr * JT * P // 2, (qtr + 1) * JT * P // 2)
                fsl = slice(qtr * FD // 2, (qtr + 1) * FD // 2)
                eng.dma_start(
                    tw[:, fsl].rearrange("p (j c) -> p j c", c=C),
                    d_[rows, :].rearrange("(j p) c -> p j c", p=P),
                )

            # k0 first on sync, q0 on scalar: these two finish before the
            # rest of the input stream starts competing for bandwidth.
            load_qtr(nc.sync, ktw, k_d, 0)
            load_qtr(nc.sync, ktw, k_d, 1)
            load_qtr(nc.scalar, qtw, q_d, 0)
            load_qtr(nc.scalar, qtw, q_d, 1)
            tri_t = cpool.tile([P, P], f16, tag="tri")
            nc.sync.dma_start(tri_t[:], tri_d[:])
            colsel_t = cpool.tile([P, JT * JT], f16, tag="colsel")
            nc.sync.dma_start(colsel_t[:], colsel_d[:])
            gate2_t = cpool.tile([NT, NT * P], f16, tag="gate2")
            nc.sync.dma_start(gate2_t[:], gate2_d[:])
            # gating dummy: reads k0's SBUF region, so this queue entry (and
            # everything behind it) waits until k0 has fully landed -- the
            # remaining input streams then can't steal bandwidth from k0/q0
            dummy_t = cpool.tile([1, 4], f32, tag="dummy")
            nc.sync.dma_start(dummy_t[:], ktw[0:1, 0:4])
            load_qtr(nc.sync, ktw, k_d, 2)
            load_qtr(nc.sync, ktw, k_d, 3)
            load_qtr(nc.sync, qtw, q_d, 2)
            load_qtr(nc.sync, qtw, q_d, 3)
            load_qtr(nc.sync, vtw, v_d, 0)
            load_qtr(nc.sync, vtw, v_d, 1)
            load_qtr(nc.sync, vtw, v_d, 2)
            load_qtr(nc.sync, vtw, v_d, 3)

            # segmented-scan reset mask: 0 at the first column of each 64-seg
            mask_t = cpool.tile([P, FD], f16, tag="mask")
            nc.gpsimd.memset(mask_t[:], 1.0)
            nc.gpsimd.memset(
                mask_t[:].rearrange("p (s d) -> p s d", d=D)[:, :, 0:1], 0.0
            )
            # per-tile colsum rows for chunk>0, spread over partitions
            csm = cpool.tile([NT, C], f16, tag="csm")
            cst = [None] * NCH

            for ch in range(NCH):
                gsl = slice(ch * FD, (ch + 1) * FD)
                kt = ktw[:, gsl]
                qt = qtw[:, gsl]
                vt = vtw[:, gsl]

                # ---- feature maps: f = min(exp(x),1) + relu(x) ----
                # (exp+relu on the scalar engine; fused min+add on DVE)
                ek = wk.tile([P, FD], f16, tag="ek")
                nc.scalar.activation(ek[:], kt, Act.Exp)
                rk = wk.tile([P, FD], f16, tag="rk")
                nc.scalar.activation(rk[:], kt, Act.Relu)
                kf = wk.tile([P, FD], f16, tag="kf")
                nc.vector.scalar_tensor_tensor(
                    kf[:], ek[:], 1.0, rk[:], op0=Alu.min, op1=Alu.add
                )

                eq = wk.tile([P, FD], f16, tag="eq")
                nc.scalar.activation(eq[:], qt, Act.Exp)
                rq = wk.tile([P, FD], f16, tag="rq")
                nc.scalar.activation(rq[:], qt, Act.Relu)
                qf = wk.tile([P, FD], f16, tag="qf")
                nc.vector.scalar_tensor_tensor(
                    qf[:], eq[:], 1.0, rq[:], op0=Alu.min, op1=Alu.add
                )

                # ---- per-tile colsums onto psum rows 0..JT-1 ----
                cs_ps = pcs.tile([JT, C], f32, tag="cs")
                for j in range(JT):
                    nc.tensor.matmul(
                        cs_ps[:],
                        colsel_t[:, j * JT : (j + 1) * JT],
                        kf[:, j * C : (j + 1) * C],
                        start=(j == 0), stop=(j == JT - 1),
                    )
                cst_ch = wk.tile([JT, C], f16, tag=f"cst{ch}")
                cst[ch] = cst_ch
                nc.scalar.copy(cst_ch[:], cs_ps[:])
                # stage rows into csm (only chunk>0 reads it)
                nc.sync.dma_start(csm[ch * JT : (ch + 1) * JT, :], cst_ch[:])

                # ---- cumsum over T into PSUM: tri matmuls + per-tile
                # gated-colsum base matmuls (start=True resets the whole
                # PSUM bank, so only the first write per bank may set it)
                nk = (ch + 1) * JT
                cs_src = cst[0] if ch == 0 else csm
                ks = pp.tile([P, FD], f32, tag="ks")
                for j in range(JT):
                    sl = slice(j * C, (j + 1) * C)
                    nc.tensor.matmul(
                        ks[:, sl], tri_t[:], kf[:, sl],
                        start=(j % 4 == 0), stop=False,
                    )
                for j in range(JT):
                    jg = ch * JT + j
                    sl = slice(j * C, (j + 1) * C)
                    if jg == 0:
                        continue
                    nc.tensor.matmul(
                        ks[:, sl],
                        gate2_t[0:nk, jg * P : (jg + 1) * P],
                        cs_src[0:nk, :],
                        start=False,
                        stop=(j % 4 == 3),
                    )

                # ---- cumsum over D: one segmented scan ----
                csDm = wk.tile([P, FD], f16, tag="csDm")
                nc.vector.tensor_tensor_scan(
                    csDm[:], mask_t[:], kf[:], 0.0, op0=Alu.mult, op1=Alu.add
                )

                # ---- dots over D (s-part first: ready before ks) ----
                scr = wk.tile([P, 2 * FD], f16, tag="scr")
                dn = wk.tile([P, 2 * NSEG], f32, tag="dn")
                nc.vector.tensor_tensor(
                    scr[:, FD : 2 * FD], qf[:], csDm[:], op=Alu.mult
                )
                nc.vector.tensor_reduce(
                    dn[:, NSEG : 2 * NSEG],
                    scr[:, FD : 2 * FD].rearrange("p (s d) -> p s d", d=D),
                    axis=mybir.AxisListType.X,
                    op=Alu.add,
                )
                nc.vector.tensor_tensor(
                    scr[:, 0:FD], qf[:], ks[:], op=Alu.mult
                )
                nc.vector.tensor_reduce(
                    dn[:, 0:NSEG],
                    scr[:, 0:FD].rearrange("p (s d) -> p s d", d=D),
                    axis=mybir.AxisListType.X,
                    op=Alu.add,
                )

                # scale = s / denom
                rec = wk.tile([P, NSEG], f32, tag="rec")
                nc.vector.reciprocal(rec[:], dn[:, 0:NSEG])
                sc = wk.tile([P, NSEG], f32, tag="sc")
                nc.vector.tensor_tensor(
                    sc[:], dn[:, NSEG : 2 * NSEG], rec[:], op=Alu.mult
                )

                # out = v * scale (broadcast over each 64-wide segment),
                # in halves so the store DMA overlaps the second half
                ot = io.tile([P, FD], f32, tag="o")
                for half in range(2):
                    hs = slice(half * FD // 2, (half + 1) * FD // 2)
                    sseg = slice(half * NSEG // 2, (half + 1) * NSEG // 2)
                    sc_b = sc[:, sseg].rearrange(
                        "p (s one) -> p s one", one=1
                    ).broadcast_to([P, NSEG // 2, D])
                    nc.vector.tensor_tensor(
                        ot[:, hs].rearrange("p (s d) -> p s d", d=D),
                        vt[:, hs].rearrange("p (s d) -> p s d", d=D),
                        sc_b,
                        op=Alu.mult,
                    )
                    qtr = ch * 2 + half
                    rows = slice(qtr * JT * P // 2, (qtr + 1) * JT * P // 2)
                    ov = o_d[rows, :].rearrange("(j p) c -> p j c", p=P)
                    nc.sync.dma_start(
                        ov, ot[:, hs].rearrange("p (j c) -> p j c", c=C)
                    )

    nc.compile()
    return nc


def get_nc():
    if "nc" not in _CACHE:
        _CACHE["nc"] = _build_nc()
    return _CACHE["nc"]


def shard_inputs(q, k, v):
    """core c -> (b = c//4, heads 2*(c%4), 2*(c%4)+1); returns list of in_maps."""
    maps = []
    for c in range(8):
        b, hp = divmod(c, 4)
        hs = slice(2 * hp, 2 * hp + 2)
        maps.append(
            {
                "q": np.ascontiguousarray(q[b, :, hs, :].reshape(T, C)),
                "k": np.ascontiguousarray(k[b, :, hs, :].reshape(T, C)),
                "v": np.ascontiguousarray(v[b, :, hs, :].reshape(T, C)),
            }
        )
    return maps


def gather_outputs(results):
    out = np.empty((B, T, H, D), dtype=np.float32)
    for c in range(8):
        b, hp = divmod(c, 4)
        out[b, :, 2 * hp : 2 * hp + 2, :] = results[c]["o"].reshape(T, HPC, D)
    return out


def kernel(q, k, v):
    from concourse.bass_utils import run_bass_kernel_spmd

    q = np.asarray(q, dtype=np.float32)
    k = np.asarray(k, dtype=np.float32)
    v = np.asarray(v, dtype=np.float32)
    nc = get_nc()
    maps = shard_inputs(q, k, v)
    res = run_bass_kernel_spmd(nc, maps, list(range(8)))
    return gather_outputs(res.results)


# revision 23
# speedup vs baseline: 1.0717x; 1.0253x over previous
"""Trainium2 Bass kernel for nn_CausalLinearSelfAttention_30013231464545.

Math note: the reference cumsums the [B,T,H,D,M] kv tensor over axis=-2,
which is the *D* axis (faithful to the original torch code), so
  kv_sum[b,t,h,d,m] = csD(kf)[b,t,h,d] * v[b,t,h,m]
and the whole module collapses to
  out[b,t,h,m] = (s / denom) * v[b,t,h,m]
with
  denom[b,t,h] = sum_d qf * cumsum_T(kf)      (true causal running key sum)
  s[b,t,h]     = sum_d qf * cumsum_D(kf)      (per-timestep D-prefix sum)
  qf = elu(q)+1 = min(exp(q), 1) + relu(q),  kf likewise.

Sharding: B*H = 16 (b,h) slices; each core takes one (b, head-pair) slice
[T=2048, 2*64] so DMA rows are 512B contiguous. No cross-core comm.

Per-core dataflow, v2 (T on partitions, (j,h,d) on free; fp16 on-chip):
  - 2 half-tensor chunks pipelined against the input DMA stream
  - feature maps: ACT exp (scalar) + fused (min 1) add relu via one
    scalar_tensor_tensor (DVE)
  - cumsum over T: per-tile triangular matmuls; the inter-tile carry runs
    entirely on the tensor engine (colsum matmuls onto partitions,
    one exclusive-tri16 matmul, rank-1 base adds) - no serial DVE chain
  - scalar engine copies PSUM ks -> SBUF f16 so the dot mults stay in
    DVE 2x fast mode
  - cumsum over D: one segmented scan per chunk (GpSimd, off the DVE)
  - dots over D: two f16 TT mults + one multi-axis tensor_reduce
  - out = v * (s/denom) broadcast multiply, DMA'd out per chunk
"""

import numpy as np
import sys

sys.path.insert(0, "/opt/trn_rl_repo")

B, T, H, D = 2, 2048, 8, 64
P = 128          # partitions (t per tile)
HPC = 2          # heads per core
C = HPC * D      # per-core free width = 128
NT = T // P      # 16 t-tiles per core
NCH = 2          # pipeline chunks
JT = NT // NCH   # 8 tiles per chunk
FD = JT * C      # 1024 free elements per chunk
NSEG = JT * HPC  # 16 (tile, head) segments per chunk

USE_GPSIMD_SCAN = False  # compiler rejects TensorTensorScan on the Pool engine

_CACHE = {}


def _build_nc():
    import concourse.bass as bass
    import concourse.bacc as bacc
    import concourse.mybir as mybir
    from concourse import tile

    dt = mybir.dt
    f32 = dt.float32
    f16 = dt.float16
    Alu = mybir.AluOpType
    Act = mybir.ActivationFunctionType

    nc = bacc.Bacc(None)

    q_d = nc.declare_dram_parameter("q", [T, C], f32, isOutput=False)
    k_d = nc.declare_dram_parameter("k", [T, C], f32, isOutput=False)
    v_d = nc.declare_dram_parameter("v", [T, C], f32, isOutput=False)
    o_d = nc.declare_dram_parameter("o", [T, C], f32, isOutput=True)

    # tri[t', t] = 1 if t' <= t  (lhsT for in-tile cumsum over partitions)
    tri_d = nc.inline_tensor(
        np.triu(np.ones((P, P), dtype=np.float16)), name="tri_const"
    )
    # gate2[p, (j, m)] = 1 if p < j: lhsT that sums colsum rows < j into
    # a broadcast over all 128 output partitions (the inter-tile base add)
    gate2_np = np.zeros((NT, NT, P), dtype=np.float16)
    for j in range(NT):
        gate2_np[:j, j, :] = 1.0
    gate2_d = nc.inline_tensor(gate2_np.reshape(NT, NT * P), name="gate2_const")
    # colsel[:, j, m] = (m == j): drops tile j's colsum onto psum row j
    colsel_np = np.zeros((P, JT, JT), dtype=np.float16)
    for j in range(JT):
        colsel_np[:, j, j] = 1.0
    colsel_d = nc.inline_tensor(colsel_np.reshape(P, JT * JT), name="colsel_const")

    with tile.TileContext(nc) as tc:
        with (
            tc.tile_pool(name="const", bufs=1) as cpool,
            tc.tile_pool(name="io", bufs=2) as io,
            tc.tile_pool(name="wk", bufs=2) as wk,
            tc.tile_pool(name="ps", bufs=2, space="PSUM") as pp,
            tc.tile_pool(name="pcs", bufs=1, space="PSUM") as pcs,
        ):
            qtw = io.tile([P, T], f32, tag="q")
            ktw = io.tile([P, T], f32, tag="k")
            vtw = io.tile([P, T], f32, tag="v")

            def load_qtr(eng, tw, d_, qtr):
                rows = slice(qtr * JT * P // 2, (qtr + 1) * JT * P // 2)
                fsl = slice(qtr * FD // 2, (qtr + 1) * FD // 2)
                eng.dma_start(
                    tw[:, fsl].rearrange("p (j c) -> p j c", c=C),
                    d_[rows, :].rearrange("(j p) c -> p j c", p=P),
                )

            # k0 striped over BOTH hwdge queues (a single queue moves only
            # ~130GB/s), then q0; later streams are gated behind the dummy.
            load_qtr(nc.sync, ktw, k_d, 0)
            load_qtr(nc.scalar, ktw, k_d, 1)
            load_qtr(nc.sync, qtw, q_d, 0)
            load_qtr(nc.scalar, qtw, q_d, 1)
            tri_t = cpool.tile([P, P], f16, tag="tri")
            nc.sync.dma_start(tri_t[:], tri_d[:])
            colsel_t = cpool.tile([P, JT * JT], f16, tag="colsel")
            nc.sync.dma_start(colsel_t[:], colsel_d[:])
            gate2_t = cpool.tile([NT, NT * P], f16, tag="gate2")
            nc.sync.dma_start(gate2_t[:], gate2_d[:])
            # gating dummy: reads k0's SBUF region, so this queue entry (and
            # everything behind it) waits until k0 has fully landed -- the
            # remaining input streams then can't steal bandwidth from k0/q0
            dummy_t = cpool.tile([1, 4], f32, tag="dummy")
            nc.sync.dma_start(dummy_t[:], ktw[0:1, 0:4])
            load_qtr(nc.sync, ktw, k_d, 2)
            load_qtr(nc.sync, ktw, k_d, 3)
            load_qtr(nc.sync, qtw, q_d, 2)
            load_qtr(nc.sync, qtw, q_d, 3)
            load_qtr(nc.sync, vtw, v_d, 0)
            load_qtr(nc.sync, vtw, v_d, 1)
            load_qtr(nc.sync, vtw, v_d, 2)
            load_qtr(nc.sync, vtw, v_d, 3)

            # segmented-scan reset mask: 0 at the first column of each 64-seg
            mask_t = cpool.tile([P, FD], f16, tag="mask")
            nc.gpsimd.memset(mask_t[:], 1.0)
            nc.gpsimd.memset(
                mask_t[:].rearrange("p (s d) -> p s d", d=D)[:, :, 0:1], 0.0
            )
            # per-tile colsum rows for chunk>0, spread over partitions
            csm = cpool.tile([NT, C], f16, tag="csm")
            cst = [None] * NCH

            for ch in range(NCH):
                gsl = slice(ch * FD, (ch + 1) * FD)
                kt = ktw[:, gsl]
                qt = qtw[:, gsl]
                vt = vtw[:, gsl]

                # ---- feature maps: f = min(exp(x),1) + relu(x) ----
                # (exp+relu on the scalar engine; fused min+add on DVE)
                ek = wk.tile([P, FD], f16, tag="ek")
                nc.scalar.activation(ek[:], kt, Act.Exp)
                rk = wk.tile([P, FD], f16, tag="rk")
                nc.scalar.activation(rk[:], kt, Act.Relu)
                kf = wk.tile([P, FD], f16, tag="kf")
                nc.vector.scalar_tensor_tensor(
                    kf[:], ek[:], 1.0, rk[:], op0=Alu.min, op1=Alu.add
                )

                eq = wk.tile([P, FD], f16, tag="eq")
                nc.scalar.activation(eq[:], qt, Act.Exp)
                rq = wk.tile([P, FD], f16, tag="rq")
                nc.scalar.activation(rq[:], qt, Act.Relu)
                qf = wk.tile([P, FD], f16, tag="qf")
                nc.vector.scalar_tensor_tensor(
                    qf[:], eq[:], 1.0, rq[:], op0=Alu.min, op1=Alu.add
                )

                # ---- per-tile colsums onto psum rows 0..JT-1 ----
                cs_ps = pcs.tile([JT, C], f32, tag="cs")
                for j in range(JT):
                    nc.tensor.matmul(
                        cs_ps[:],
                        colsel_t[:, j * JT : (j + 1) * JT],
                        kf[:, j * C : (j + 1) * C],
                        start=(j == 0), stop=(j == JT - 1),
                    )
                cst_ch = wk.tile([JT, C], f16, tag=f"cst{ch}")
                cst[ch] = cst_ch
                nc.scalar.copy(cst_ch[:], cs_ps[:])
                # stage rows into csm (only chunk>0 reads it)
                nc.sync.dma_start(csm[ch * JT : (ch + 1) * JT, :], cst_ch[:])

                # ---- cumsum over T into PSUM: tri matmuls + per-tile
                # gated-colsum base matmuls (start=True resets the whole
                # PSUM bank, so only the first write per bank may set it)
                nk = (ch + 1) * JT
                cs_src = cst[0] if ch == 0 else csm
                ks = pp.tile([P, FD], f32, tag="ks")
                for j in range(JT):
                    sl = slice(j * C, (j + 1) * C)
                    nc.tensor.matmul(
                        ks[:, sl], tri_t[:], kf[:, sl],
                        start=(j % 4 == 0), stop=False,
                    )
                for j in range(JT):
                    jg = ch * JT + j
                    sl = slice(j * C, (j + 1) * C)
                    if jg == 0:
                        continue
                    nc.tensor.matmul(
                        ks[:, sl],
                        gate2_t[0:nk, jg * P : (jg + 1) * P],
                        cs_src[0:nk, :],
                        start=False,
                        stop=(j % 4 == 3),
                    )

                # ---- cumsum over D: one segmented scan ----
                csDm = wk.tile([P, FD], f16, tag="csDm")
                nc.vector.tensor_tensor_scan(
                    csDm[:], mask_t[:], kf[:], 0.0, op0=Alu.mult, op1=Alu.add
                )

                # ---- dots over D (s-part first: ready before ks) ----
                scr = wk.tile([P, 2 * FD], f16, tag="scr")
                dn = wk.tile([P, 2 * NSEG], f32, tag="dn")
                nc.vector.tensor_tensor(
                    scr[:, FD : 2 * FD], qf[:], csDm[:], op=Alu.mult
                )
                nc.vector.tensor_reduce(
                    dn[:, NSEG : 2 * NSEG],
                    scr[:, FD : 2 * FD].rearrange("p (s d) -> p s d", d=D),
                    axis=mybir.AxisListType.X,
                    op=Alu.add,
                )
                nc.vector.tensor_tensor(
                    scr[:, 0:FD], qf[:], ks[:], op=Alu.mult
                )
                nc.vector.tensor_reduce(
                    dn[:, 0:NSEG],
                    scr[:, 0:FD].rearrange("p (s d) -> p s d", d=D),
                    axis=mybir.AxisListType.X,
                    op=Alu.add,
                )

                # scale = s / denom
                rec = wk.tile([P, NSEG], f32, tag="rec")
                nc.vector.reciprocal(rec[:], dn[:, 0:NSEG])
                sc = wk.tile([P, NSEG], f32, tag="sc")
                nc.vector.tensor_tensor(
                    sc[:], dn[:, NSEG : 2 * NSEG], rec[:], op=Alu.mult
                )

                # out = v * scale (broadcast over each 64-wide segment),
                # in halves so the store DMA overlaps the second half
                ot = io.tile([P, FD], f32, tag="o")
                for half in range(2):
                    hs = slice(half * FD // 2, (half + 1) * FD // 2)
                    sseg = slice(half * NSEG // 2, (half + 1) * NSEG // 2)
                    sc_b = sc[:, sseg].rearrange(
                        "p (s one) -> p s one", one=1
                    ).broadcast_to([P, NSEG // 2, D])
                    nc.vector.tensor_tensor(
                        ot[:, hs].rearrange("p (s d) -> p s d", d=D),
                        vt[:, hs].rearrange("p (s d) -> p s d", d=D),
                        sc_b,
                        op=Alu.mult,
                    )
                    qtr = ch * 2 + half
                    rows = slice(qtr * JT * P // 2, (qtr + 1) * JT * P // 2)
                    ov = o_d[rows, :].rearrange("(j p) c -> p j c", p=P)
                    oeng = nc.sync if half == 0 else nc.scalar
                    oeng.dma_start(
                        ov, ot[:, hs].rearrange("p (j c) -> p j c", c=C)
                    )

    nc.compile()
    return nc


def get_nc():
    if "nc" not in _CACHE:
        _CACHE["nc"] = _build_nc()
    return _CACHE["nc"]


def shard_inputs(q, k, v):
    """core c -> (b = c//4, heads 2*(c%4), 2*(c%4)+1); returns list of in_maps."""
    maps = []
    for c in range(8):
        b, hp = divmod(c, 4)
        hs = slice(2 * hp, 2 * hp + 2)
        maps.append(
            {
                "q": np.ascontiguousarray(q[b, :, hs, :].reshape(T, C)),
                "k": np.ascontiguousarray(k[b, :, hs, :].reshape(T, C)),
                "v": np.ascontiguousarray(v[b, :, hs, :].reshape(T, C)),
            }
        )
    return maps


def gather_outputs(results):
    out = np.empty((B, T, H, D), dtype=np.float32)
    for c in range(8):
        b, hp = divmod(c, 4)
        out[b, :, 2 * hp : 2 * hp + 2, :] = results[c]["o"].reshape(T, HPC, D)
    return out


def kernel(q, k, v):
    from concourse.bass_utils import run_bass_kernel_spmd

    q = np.asarray(q, dtype=np.float32)
    k = np.asarray(k, dtype=np.float32)
    v = np.asarray(v, dtype=np.float32)
    nc = get_nc()
    maps = shard_inputs(q, k, v)
    res = run_bass_kernel_spmd(nc, maps, list(range(8)))
    return gather_outputs(res.results)
